# revision 47
# baseline (speedup 1.0000x reference)
"""TRN2 Bass kernel for nn_Codec (VQ autoencoder), 8-way data-parallel over batch.

Contract: kernel(**inputs) takes the FULL unsharded inputs (as produced by
setup_inputs()) and returns the FULL [4096, 3, 32, 32] float32 output.

Design (per core, batch slice of 512 rows, activations kept transposed so the
batch lives on the free dimension and weight tiles stream straight from DRAM):
  xT [3072,512] --GEMM1 f16--> hT --tanh--> GEMM2 f16 (PSUM-held accum)
  --> h2T [256,512] --tanh--> VQ --> yT [256,512] bf16 --decoder GEMM1 bf16-->
  y2T [16384,512] bf16 --decoder GEMM2 bf16--> outT [3072,512] --> host
  gathers the 8 column blocks and transposes back.

VQ (per 128-row batch tile, all 4 chunks batched):
  d^2 for all 4 chunks comes from one K=66 augmented matmul per chunk
  (rows: h2 chunk, ones->+c2, x2->+x2; x2 itself is a tiny ones^T @ h2^2
  f32r matmul), so no batch-major transposes of h2 are needed.  Then
  sqrt+exp on ACT (phase-ordered to avoid table thrash), grouped reductions
  on DVE for softmax sums / row maxima, the jax threefry noise (precomputed
  bit-exact on host) subtracted on GpSimd, and argmax via is_equal*iota +
  grouped max.  The one-hot gather matmuls run in bf16 (yT is bf16 anyway).

x and the noise tables are DMA'd as single tiles with 16-24KB contiguous
per-partition lines (full HBM rate; split [128,512] tiles only reach
~1/3 rate on 1KB packets), on the Scalar DGE queue so they overlap the
weight stream on the Sync queue.
"""
import os
import sys
from contextlib import ExitStack

import numpy as np

for _p in ("/opt/trn_rl_repo", "/root/.axon_site/_ro/trn_rl_repo"):
    if os.path.isdir(_p) and _p not in sys.path:
        sys.path.append(_p)

import concourse.bass as bass  # noqa: E402
import concourse.tile as tile  # noqa: E402
from concourse import bacc, mybir  # noqa: E402
from concourse.bass_utils import run_bass_kernel_spmd  # noqa: E402

F32 = mybir.dt.float32
F32R = mybir.dt.float32r
BF16 = mybir.dt.bfloat16
F16 = mybir.dt.float16
F8E4 = mybir.dt.float8e4
U32 = mybir.dt.uint32
I32 = mybir.dt.int32
AF = mybir.ActivationFunctionType
ALU = mybir.AluOpType
AX = mybir.AxisListType

N_CORES = 8
BTOT = 4096
B = BTOT // N_CORES          # 512 batch rows per core
IMG = 3 * 32 * 32            # 3072
HID = 16384
NCODE = 256
DCODE = 64
BT = B // 128
KT1 = IMG // 128
MT1 = HID // 128
NT2 = IMG // 128
# decoder GEMM2: first KF of the HID reduction runs in fp8 DoubleRow
# (2x K per matmul, ~1.44x faster); numpy-validated rel err 1.79e-2 < 2e-2.
KF = 4096
MTF = KF // 128              # 32 fp8 k-tiles
PF = MTF // 2                # 16 DoubleRow k-tile pairs
W8SCALE = 2048.0             # fp8 weight scale (undone in the bias/copy step)


# ---------------------------------------------------------------------------
# numpy reimplementation of jax threefry2x32 noise (bit-exact, partitionable)
# ---------------------------------------------------------------------------
def _rotl(x, r):
    return ((x << np.uint32(r)) | (x >> np.uint32(32 - r))) & np.uint32(0xFFFFFFFF)


def _threefry_core(key, x0, x1):
    ks0, ks1 = np.uint32(key[0]), np.uint32(key[1])
    ks2 = np.uint32(ks0 ^ ks1 ^ np.uint32(0x1BD11BDA))
    rotations = [(13, 15, 26, 6), (17, 29, 16, 24)]
    x0 = (x0 + ks0).astype(np.uint32)
    x1 = (x1 + ks1).astype(np.uint32)
    ks = [ks1, ks2, ks0, ks1, ks2, ks0]
    for i in range(5):
        for r in rotations[i % 2]:
            x0 = (x0 + x1).astype(np.uint32)
            x1 = _rotl(x1, r)
            x1 = (x0 ^ x1).astype(np.uint32)
        x0 = (x0 + ks[i]).astype(np.uint32)
        x1 = (x1 + ks[i + 1] + np.uint32(i + 1)).astype(np.uint32)
    return x0, x1


def _fold_in(key, data):
    return _threefry_core(key, np.array([0], np.uint32),
                          np.array([data], np.uint32))


def _uniform_f32(key, n):
    b0, b1 = _threefry_core(key, np.zeros(n, np.uint32),
                            np.arange(n, dtype=np.uint32))
    bits = (b0 ^ b1).astype(np.uint32)
    f = ((bits >> np.uint32(9)) | np.uint32(0x3F800000)).view(np.float32)
    return f - np.float32(1.0)


def _noise_tables():
    key = np.array([0, 12345], dtype=np.uint32)  # jax.random.key(12345)
    out = []
    for i in (1, 2, 3, 4):
        k = _fold_in(key, i)
        k = np.array([k[0][0], k[1][0]], np.uint32)
        out.append(_uniform_f32(k, BTOT * NCODE).reshape(BTOT, NCODE))
    return np.stack(out)  # [4, BTOT, 256]


# ---------------------------------------------------------------------------
# kernel builder
# ---------------------------------------------------------------------------
def _build_kernel():
    nc = bacc.Bacc("TRN2", target_bir_lowering=False, debug=False)

    xp_d = nc.dram_tensor("xpack", [128, KT1, B], F16, kind="ExternalInput").ap()
    w1e_d = nc.dram_tensor("w1e", [MT1, 128, IMG], F16, kind="ExternalInput").ap()
    w2e_d = nc.dram_tensor("w2e", [MT1, 128, NCODE], F16, kind="ExternalInput").ap()
    b1e_d = nc.dram_tensor("b1e", [128, MT1], F32, kind="ExternalInput").ap()
    b2e_d = nc.dram_tensor("b2e", [128, 2], F32, kind="ExternalInput").ap()
    cbaug_d = nc.dram_tensor("cbaug", [66, NCODE], F32, kind="ExternalInput").ap()
    x2sel_d = nc.dram_tensor("x2sel", [DCODE + 1, 2], F16, kind="ExternalInput").ap()
    cbk_d = nc.dram_tensor("cbk", [2, 128, DCODE], BF16, kind="ExternalInput").ap()
    u_d = nc.dram_tensor("upack", [128, 4 * BT, NCODE], F32, kind="ExternalInput").ap()
    w1d_d = nc.dram_tensor("w1d", [MT1 // 4, 128, 4 * NCODE], BF16, kind="ExternalInput").ap()
    b1d_d = nc.dram_tensor("b1d", [128, MT1], F32, kind="ExternalInput").ap()
    w2d8_d = nc.dram_tensor("w2d8", [NT2, 128, KF], F8E4, kind="ExternalInput").ap()
    w2d_d = nc.dram_tensor("w2d", [NT2, 128, HID - KF], BF16, kind="ExternalInput").ap()
    b2d_d = nc.dram_tensor("b2d", [128, NT2], F32, kind="ExternalInput").ap()
    outT_d = nc.dram_tensor("outT", [NT2, 128, B], F32, kind="ExternalOutput").ap()

    with tile.TileContext(nc) as tc, ExitStack() as octx:
        const_pool = octx.enter_context(tc.tile_pool(name="const", bufs=1))
        yT_pool = octx.enter_context(tc.tile_pool(name="yT", bufs=2))

        # h2 chunks live in augmented [66, B] tiles: rows 0-63 chunk dims,
        # row 64 = x2 (pairs with the ones row of cbaug), row 65 = 1.0
        # (pairs with the c2 row of cbaug).  They (and the noise tables) are
        # only needed through the VQ phase; the scope frees them before the
        # decoder's SBUF peak.  Entered before xp_scope so pools pop LIFO.
        vq_scope = octx.enter_context(ExitStack())
        vq_pool = vq_scope.enter_context(tc.tile_pool(name="vqp", bufs=1))
        h2aug = [vq_pool.tile([66, B], F32, tag=f"h2aug{c}", name=f"h2aug{c}")
                 for c in range(4)]
        u_sb = vq_pool.tile([128, 4 * BT, NCODE], F32)

        # x first in the Sync queue's FIFO: the first matmul chain needs the
        # whole packed x tile, everything else can land behind it.  Two
        # halves so the k-chain can start on the first half.
        xp_scope = octx.enter_context(ExitStack())
        xppool = xp_scope.enter_context(tc.tile_pool(name="xp", bufs=1))
        xpack = xppool.tile([128, KT1, B], F16)
        nc.sync.dma_start(xpack[:, 0:KT1 // 2], xp_d[:, 0:KT1 // 2])

        # --- constants (Sync queue; scheduled clear of the x/w1 ramp where
        # they are not needed until later) ---
        b1e_sb = const_pool.tile([128, MT1], F32)
        nc.sync.dma_start(b1e_sb[:], b1e_d[:])
        b2e_sb = const_pool.tile([128, 2], F32)
        nc.sync.dma_start(b2e_sb[:], b2e_d[:])
        with tc.tile_wait_until(0.06):
            b1d_sb = const_pool.tile([128, MT1], F32)
            nc.sync.dma_start(b1d_sb[:], b1d_d[:])
            b2d_sb = const_pool.tile([128, NT2], F32)
            nc.sync.dma_start(b2d_sb[:], b2d_d[:])
            cbaug_sb = const_pool.tile([66, NCODE], F32)
            nc.sync.dma_start(cbaug_sb[:], cbaug_d[:])
        cbk_sb = []
        for pt in range(2):
            t = const_pool.tile([128, DCODE], BF16, tag=f"cbk{pt}", name=f"cbk{pt}")
            nc.sync.dma_start(t[:], cbk_d[pt])
            cbk_sb.append(t)
        # x2sel^T @ [sq; ones] -> [x2; ones] (2 rows, copyable at offset 64);
        # f16 so the matmul is single-pass (numpy-validated flip-free)
        x2sel = const_pool.tile([DCODE + 1, 2], F16)
        nc.sync.dma_start(x2sel[:], x2sel_d[:])
        ones_row = const_pool.tile([1, 128], F16)
        nc.vector.memset(ones_row[:], 1.0)
        iota_i = const_pool.tile([128, 1], I32)
        nc.gpsimd.iota(iota_i[:], [[0, 1]], base=0, channel_multiplier=1)
        iota_col = []
        for pt in range(2):
            t = const_pool.tile([128, 1], F32, tag=f"iotac{pt}", name=f"iotac{pt}")
            if pt == 0:
                nc.vector.tensor_copy(t[:], iota_i[:])
            else:
                nc.vector.tensor_scalar_add(t[:], iota_col[0][:], float(128))
            iota_col.append(t)
        iota_row_i = const_pool.tile([128, NCODE], I32)
        nc.gpsimd.iota(iota_row_i[:], [[1, NCODE]], base=0, channel_multiplier=0)
        iota_row_f = const_pool.tile([128, NCODE], F32)
        nc.vector.tensor_copy(iota_row_f[:], iota_row_i[:])
        ident = const_pool.tile([128, 128], F32)
        nc.vector.tensor_scalar(ident[:], iota_row_f[:, 0:128], iota_col[0][:], None,
                                op0=ALU.is_equal)

        # ---------------- encoder ----------------
        with ExitStack() as ctx:
            w1pool = ctx.enter_context(tc.tile_pool(name="w1", bufs=4))
            w2pool = ctx.enter_context(tc.tile_pool(name="w2", bufs=3))
            hpool = ctx.enter_context(tc.tile_pool(name="h", bufs=4))
            sqpool = ctx.enter_context(tc.tile_pool(name="sq", bufs=1))
            gpsum = ctx.enter_context(tc.tile_pool(name="gps", bufs=3, space="PSUM"))
            h2psum = ctx.enter_context(tc.tile_pool(name="h2ps", bufs=1, space="PSUM"))
            x2psum = ctx.enter_context(tc.tile_pool(name="x2ps", bufs=2, space="PSUM"))

            w1_first = w1pool.tile([128, IMG], F16, tag="w1", name="w1_first")
            nc.sync.dma_start(w1_first[:], w1e_d[0])
            nc.sync.dma_start(xpack[:, KT1 // 2:], xp_d[:, KT1 // 2:])

            h2t_ps = [h2psum.tile([128, B], F32, tag=f"h2t{i}", name=f"h2t{i}")
                      for i in range(2)]

            scratch1 = hpool.tile([128, 1], F32, tag="scr", name="scratch1")
            prev = None
            for mt in range(MT1):
                if mt == 24:
                    # noise tables: Scalar DGE queue, scheduled well clear of
                    # the ramp (the tile scheduler reorders by dependency, so
                    # an explicit wait keeps it from competing with x/w1)
                    with tc.tile_wait_until(0.25):
                        nc.scalar.dma_start(u_sb[:], u_d[:])
                if mt == 64:
                    # preload the sqrt ACT table into the spare slot during
                    # encoder slack so the VQ phase skips one table load
                    nc.scalar.activation(scratch1[:], b1e_sb[:, 0:1], AF.Sqrt)
                if mt == 0:
                    w1 = w1_first
                else:
                    w1 = w1pool.tile([128, IMG], F16, tag="w1")
                    nc.sync.dma_start(w1[:], w1e_d[mt])
                w2 = w2pool.tile([128, NCODE], F16, tag="w2")
                nc.sync.dma_start(w2[:], w2e_d[mt])
                p = gpsum.tile([128, B], F32, tag="gp")
                for kt in range(KT1):
                    nc.tensor.matmul(p[:], w1[:, kt * 128:(kt + 1) * 128],
                                     xpack[:, kt], start=(kt == 0),
                                     stop=(kt == KT1 - 1))
                ht = hpool.tile([128, B], F16, tag="ht")
                nc.scalar.activation(ht[:], p[:], AF.Tanh, bias=b1e_sb[:, mt:mt + 1])
                if prev is not None:
                    pw2, pht, pmt = prev
                    for i in range(2):
                        nc.tensor.matmul(h2t_ps[i][:], pw2[:, i * 128:(i + 1) * 128],
                                         pht[:], start=(pmt == 0), stop=False)
                prev = (w2, ht, mt)
            pw2, pht, pmt = prev
            for i in range(2):
                nc.tensor.matmul(h2t_ps[i][:], pw2[:, i * 128:(i + 1) * 128], pht[:],
                                 start=(pmt == 0), stop=True)
            # tanh straight into the augmented chunk tiles
            for c in range(4):
                i, half = c // 2, (c % 2) * DCODE
                nc.scalar.activation(
                    h2aug[c][0:DCODE, :], h2t_ps[i][half:half + DCODE, :], AF.Tanh,
                    bias=b2e_sb[half:half + DCODE, i:i + 1])
            # rows 64-65 of h2aug: [x2; ones] via a tiny f32r select-matmul
            # over [sq; ones] (validated flip-free in numpy)
            for c in range(4):
                sq = sqpool.tile([DCODE + 1, B], F16, tag=f"sq{c}", name=f"sq{c}")
                nc.vector.scalar_tensor_tensor(sq[0:DCODE, :], h2aug[c][0:DCODE, :],
                                               1.0, h2aug[c][0:DCODE, :],
                                               op0=ALU.mult, op1=ALU.mult)
                nc.vector.memset(sq[DCODE:DCODE + 1, :], 1.0)
                x2p = x2psum.tile([2, B], F32, tag="x2p")
                nc.tensor.matmul(x2p[:], x2sel[:], sq[:], start=True, stop=True)
                nc.vector.tensor_copy(h2aug[c][64:66, :], x2p[:])

        xp_scope.close()  # x tile dead once the encoder is done

        # ---------------- VQ ----------------
        yT_sb = [yT_pool.tile([128, B], BF16, tag=f"yT{i}", name=f"yT{i}")
                 for i in range(2)]
        with ExitStack() as ctx:
            dpool = ctx.enter_context(tc.tile_pool(name="dw", bufs=2))
            spool = ctx.enter_context(tc.tile_pool(name="small", bufs=2))
            ohpool = ctx.enter_context(tc.tile_pool(name="oh", bufs=3))
            vps = ctx.enter_context(tc.tile_pool(name="vps", bufs=2, space="PSUM"))
            pps = ctx.enter_context(tc.tile_pool(name="pps", bufs=1, space="PSUM"))
            yps = ctx.enter_context(tc.tile_pool(name="yps", bufs=2, space="PSUM"))
            ips = ctx.enter_context(tc.tile_pool(name="ips", bufs=1, space="PSUM"))

            # distance matmuls first so the PE runs ahead of ACT/DVE
            pd_ps = []
            for bt in range(BT):
                pd = vps.tile([128, 4, NCODE], F32, tag="pd")
                for c in range(4):
                    nc.tensor.matmul(pd[:, c], h2aug[c][:, bt * 128:(bt + 1) * 128],
                                     cbaug_sb[:], start=True, stop=True)
                pd_ps.append(pd)

            # ACT per bt: sqrt then per-chunk exps (softmax sums from the ACT
            # accumulator).  sqrt and exp occupy the two ACT table slots, so
            # interleaving loads each table once and lets bt0's chain start
            # as soon as its distances are ready.
            d_sb, e_sb, s_sb = [], [], []
            for bt in range(BT):
                dt_ = dpool.tile([128, 4, NCODE], F32, tag=f"d{bt}", name=f"d{bt}")
                nc.scalar.activation(dt_[:], pd_ps[bt][:], AF.Sqrt)
                d_sb.append(dt_)
                et = dpool.tile([128, 4, NCODE], F32, tag=f"e{bt}", name=f"e{bt}")
                st = spool.tile([128, 4], F32, tag=f"s{bt}", name=f"s{bt}")
                for c in range(4):
                    nc.scalar.activation(et[:, c], dt_[:, c], AF.Exp,
                                         scale=-0.125, accum_out=st[:, c:c + 1])
                e_sb.append(et)
                s_sb.append(st)

            idxT = [spool.tile([1, B], F16, tag=f"idxT{c}", name=f"idxT{c}")
                    for c in range(4)]
            for bt in range(BT):
                s = s_sb[bt]
                rinv = spool.tile([128, 4], F32, tag="rinv")
                nc.vector.reciprocal(rinv[:], s[:])
                ru = dpool.tile([128, 4, NCODE], F32, tag="ru")
                for c in range(4):
                    nc.vector.scalar_tensor_tensor(
                        ru[:, c], e_sb[bt][:, c], rinv[:, c:c + 1],
                        u_sb[:, bt * 4 + c], op0=ALU.mult, op1=ALU.subtract)
                rmx = spool.tile([128, 4], F32, tag="rmx")
                nc.vector.tensor_reduce(rmx[:], ru[:], axis=AX.X, op=ALU.max)
                mi = dpool.tile([128, 4, NCODE], F32, tag="mi")
                for c in range(4):
                    nc.vector.scalar_tensor_tensor(
                        mi[:, c], ru[:, c], rmx[:, c:c + 1], iota_row_f[:],
                        op0=ALU.is_equal, op1=ALU.mult)
                idxf = spool.tile([128, 4], F32, tag="idxf")
                nc.vector.tensor_reduce(idxf[:], mi[:], axis=AX.X, op=ALU.max)
                for c in range(4):
                    pidx = pps.tile([1, 128], F32, tag="pidx")
                    nc.tensor.transpose(pidx[:], idxf[:, c:c + 1], ident[:])
                    nc.vector.tensor_copy(idxT[c][0:1, bt * 128:(bt + 1) * 128],
                                          pidx[:])

            for c in range(4):
                # broadcast the index row across partitions on the (idle) PE:
                # ones^T[1,128] @ idxT[1,B] -> [128,B] (indices 0-255 are
                # exact in f16)
                idxb = ips.tile([128, B], F32, tag="idxb")
                nc.tensor.matmul(idxb[:], ones_row[:], idxT[c][:],
                                 start=True, stop=True)
                yp = yps.tile([DCODE, B], F32, tag="yp")
                for pt in range(2):
                    oh = ohpool.tile([128, B], BF16, tag="oh")
                    nc.vector.tensor_scalar(oh[:], idxb[:], iota_col[pt][:], None,
                                            op0=ALU.is_equal)
                    nc.tensor.matmul(yp[:], cbk_sb[pt][:], oh[:],
                                     start=(pt == 0), stop=(pt == 1))
                nc.vector.tensor_copy(
                    yT_sb[c // 2][(c % 2) * DCODE:(c % 2 + 1) * DCODE, :], yp[:])

        vq_scope.close()  # free h2aug + noise tables before the decoder peak

        # ---------------- decoder ----------------
        with ExitStack() as ctx:
            w1dpool = ctx.enter_context(tc.tile_pool(name="w1d", bufs=3))
            y2pool = ctx.enter_context(tc.tile_pool(name="y2", bufs=1))
            w28pool = ctx.enter_context(tc.tile_pool(name="w28", bufs=3))
            w2dpool = ctx.enter_context(tc.tile_pool(name="w2d", bufs=3))
            opool = ctx.enter_context(tc.tile_pool(name="osb", bufs=3))
            dps = ctx.enter_context(tc.tile_pool(name="dps", bufs=3, space="PSUM"))
            eps = ctx.enter_context(tc.tile_pool(name="eps", bufs=2, space="PSUM"))

            # k < KF of y2 lives in fp8 (consumed by DoubleRow matmuls),
            # the rest in bf16
            y2f8 = y2pool.tile([128, PF, 2, B], F8E4)
            y2T = y2pool.tile([128, (MT1 - MTF) * B], BF16)
            for mg in range(MT1 // 4):
                w1d_sb = w1dpool.tile([128, 4 * NCODE], BF16, tag="w1d")
                nc.sync.dma_start(w1d_sb[:], w1d_d[mg])
                for ml in range(4):
                    mt = mg * 4 + ml
                    p = dps.tile([128, B], F32, tag="dp")
                    nc.tensor.matmul(p[:], w1d_sb[:, ml * 256:ml * 256 + 128],
                                     yT_sb[0][:], start=True, stop=False)
                    nc.tensor.matmul(p[:], w1d_sb[:, ml * 256 + 128:ml * 256 + 256],
                                     yT_sb[1][:], start=False, stop=True)
                    if mt < MTF:
                        nc.scalar.activation(y2f8[:, mt // 2, mt % 2, :], p[:],
                                             AF.Tanh, bias=b1d_sb[:, mt:mt + 1])
                    else:
                        mo = mt - MTF
                        nc.scalar.activation(y2T[:, mo * B:(mo + 1) * B], p[:],
                                             AF.Tanh, bias=b1d_sb[:, mt:mt + 1])

            KQ = 3
            KQL = (MT1 - MTF) // KQ
            for nt in range(NT2):
                po = eps.tile([128, B], F32, tag="ep")
                w28_sb = w28pool.tile([128, PF, 2, 128], F8E4, tag="w28")
                nc.sync.dma_start(w28_sb[:], w2d8_d[nt])
                for t in range(PF):
                    nc.tensor.matmul(po[:], w28_sb[:, t], y2f8[:, t],
                                     start=(t == 0), stop=False,
                                     perf_mode=mybir.MatmulPerfMode.DoubleRow)
                for kq in range(KQ):
                    w2d_sb = w2dpool.tile([128, KQL * 128], BF16, tag="w2d")
                    nc.sync.dma_start(
                        w2d_sb[:], w2d_d[nt, :, kq * KQL * 128:(kq + 1) * KQL * 128])
                    for kk in range(KQL):
                        kt = kq * KQL + kk
                        nc.tensor.matmul(po[:], w2d_sb[:, kk * 128:(kk + 1) * 128],
                                         y2T[:, kt * B:(kt + 1) * B],
                                         start=False, stop=(kt == MT1 - MTF - 1))
                osb = opool.tile([128, B], F32, tag="osb")
                nc.scalar.activation(osb[:], po[:], AF.Identity,
                                     bias=b2d_sb[:, nt:nt + 1], scale=1.0 / W8SCALE)
                nc.sync.dma_start(outT_d[nt], osb[:])

    nc.compile()
    return nc


def _to_bf16(x):
    import ml_dtypes
    return np.asarray(x, np.float32).astype(ml_dtypes.bfloat16)


def _prepare_in_maps(x, wb1e, wb2e, wb1d, wb2d, cb, noise_level, noises):
    W1e, b1e = wb1e[:-1], wb1e[-1]
    W2e, b2e = wb2e[:-1], wb2e[-1]
    W1d, b1d = wb1d[:-1], wb1d[-1]
    W2d, b2d = wb2d[:-1], wb2d[-1]

    xT = np.ascontiguousarray(x.T)  # [IMG, BTOT]
    w1e_p = np.ascontiguousarray(
        W1e.reshape(KT1, 128, MT1, 128).transpose(2, 1, 0, 3)).reshape(
            MT1, 128, IMG).astype(np.float16)
    w2e_p = np.ascontiguousarray(W2e.reshape(MT1, 128, NCODE)).astype(np.float16)
    b1e_p = np.ascontiguousarray(b1e.reshape(MT1, 128).T)
    b2e_p = np.ascontiguousarray(b2e.reshape(2, 128).T)
    cbaug = np.concatenate([
        (-2.0 * cb.T).astype(np.float32),
        np.ones((1, NCODE), np.float32),
        (cb * cb).sum(1, dtype=np.float32)[None, :]], axis=0)
    cbk = _to_bf16(cb.reshape(2, 128, DCODE))
    w1d_p = _to_bf16(np.ascontiguousarray(
        W1d.reshape(2, 128, MT1, 128).transpose(2, 1, 0, 3)).reshape(MT1, 128, NCODE))
    w1d_p = np.ascontiguousarray(
        w1d_p.reshape(MT1 // 4, 4, 128, NCODE).transpose(0, 2, 1, 3)).reshape(
            MT1 // 4, 128, 4 * NCODE)
    b1d_p = np.ascontiguousarray(b1d.reshape(MT1, 128).T)
    w2d_full = np.ascontiguousarray(
        W2d.reshape(MT1, 128, NT2, 128).transpose(2, 1, 0, 3)).reshape(
            NT2, 128, HID).astype(np.float32) * np.float32(W8SCALE)
    import ml_dtypes
    w2d8_p = np.clip(w2d_full[:, :, :KF], -240.0, 240.0).astype(
        ml_dtypes.float8_e4m3fn)
    w2d_p = w2d_full[:, :, KF:].astype(ml_dtypes.bfloat16)
    b2d_p = np.ascontiguousarray(b2d.reshape(NT2, 128).T)
    u_all = (np.float32(noise_level) * noises).astype(np.float32)

    x2sel = np.zeros((DCODE + 1, 2), np.float16)
    x2sel[0:DCODE, 0] = 1.0
    x2sel[DCODE, 1] = 1.0
    shared = {
        "w1e": w1e_p, "w2e": w2e_p, "b1e": b1e_p, "b2e": b2e_p,
        "cbaug": cbaug, "cbk": cbk, "x2sel": x2sel,
        "w1d": w1d_p, "b1d": b1d_p, "w2d8": w2d8_p, "w2d": w2d_p, "b2d": b2d_p,
    }
    in_maps = []
    for cix in range(N_CORES):
        sl = slice(cix * B, (cix + 1) * B)
        m = dict(shared)
        m["xpack"] = np.ascontiguousarray(
            xT[:, sl].reshape(KT1, 128, B).transpose(1, 0, 2)).astype(np.float16)
        m["upack"] = np.ascontiguousarray(
            u_all[:, sl, :].reshape(4, BT, 128, NCODE).transpose(2, 1, 0, 3)
        ).reshape(128, 4 * BT, NCODE)
        in_maps.append(m)
    return in_maps


_CACHE = {}


def kernel(x, wb1_encoder, wb2_encoder, wb1_decoder, wb2_decoder,
           codebook1, codebook2, codebook3, codebook4, noise_level,
           **_unused):
    x2d = np.ascontiguousarray(np.asarray(x, np.float32).reshape(BTOT, IMG))
    if "nc" not in _CACHE:
        _CACHE["nc"] = _build_kernel()
        _CACHE["noises"] = _noise_tables()
    nc = _CACHE["nc"]
    in_maps = _prepare_in_maps(
        x2d, np.asarray(wb1_encoder, np.float32), np.asarray(wb2_encoder, np.float32),
        np.asarray(wb1_decoder, np.float32), np.asarray(wb2_decoder, np.float32),
        np.asarray(codebook1, np.float32), float(np.asarray(noise_level)),
        _CACHE["noises"])
    res = run_bass_kernel_spmd(nc, in_maps, list(range(N_CORES)))
    cols = [r["outT"].reshape(IMG, B) for r in res.results]
    outT = np.concatenate(cols, axis=1)
    return np.ascontiguousarray(outT.T).reshape(BTOT, 3, 32, 32).astype(np.float32)
